# revision 28
# baseline (speedup 1.0000x reference)
"""Trainium2 Bass kernel for DepthLossForImgBEV (weighted one-hot depth BCE).

Math: with x = raw logits (B,N,D,H,W), gt = depth_gt (B,N,H,W):
  bce(x, t) = softplus(x) - t*x          (t = one-hot(idx); the -100 clamp in
                                          the reference never fires for |x|<100)
  loss = 3.0 * sum_{valid px} [ sum_d softplus(x) - x[idx] ] / (B*N*D*H*W)

Device computation per core (shard = 8 of 64 H-rows):
  - layout: partitions = (16 depth-bins x 8 h-rows), free = (12 cameras x 176 w)
  - pass 1 (DVE): xm = x + bigw   (bigw = -30000 at invalid pixels, 0 else)
  - pass 2 (ACT): softplus(xm) = ln(1+exp(xm)) with fused accum_out
                  (invalid pixels give softplus(-big) = 0)
  - pass 3 (DVE): scalar_tensor_tensor (gtc == d) * xm with fused accum_out
                  (gtc = bucket idx, or 200 at invalid pixels -> no match)
  - epilogue: cols1 - cols2, reduce to (128,1), DMA out.
Host: sum the 8 per-core (128,1) partials, scale by 3/numel.
"""

import numpy as np

B, N, D, H, W = 2, 6, 112, 64, 176
M = 8  # cores
HSH = H // M  # 8 h-rows per core
DD = 16  # depth bins per partition block
NT = D // DD  # 7 tiles
BN = B * N  # 12
P = 128
FREE = BN * W  # 2112
NUMEL = B * N * D * H * W
INVALID_IDX = 120.0  # any value outside [0,112]; fits int8
# mask offset: large enough that ln(1+exp(x+BIG_NEG)) == 0 exactly in f32
# (1 + 2e-33 rounds to 1.0), small enough to stay inside the ACT exp LUT's
# valid input range (~[-87, 88]) — a -30000 mask hit LUT wraparound garbage
BIG_NEG = -80.0

_CACHE = {}


def _build_bass(softplus_mode="exp_ln", mask_mode="dve_add", reps=1):
    from contextlib import ExitStack

    import concourse.bass as bass
    import concourse.mybir as mybir
    import concourse.tile as tile

    f32 = mybir.dt.float32
    nc = bass.Bass()

    i8 = mybir.dt.int8
    x = nc.declare_dram_parameter("x", [B, N * D, HSH, W], f32, isOutput=False)
    # meta[p] = [gtc (FREE) | bigw (FREE)] in int8 (all values exact),
    # host pre-replicated across the 16 depth-bin partition blocks
    meta = nc.declare_dram_parameter("meta", [P, 2 * FREE], i8, isOutput=False)
    dcol = nc.declare_dram_parameter("dcol", [P, NT], f32, isOutput=False)
    out = nc.declare_dram_parameter("out", [P, 1], f32, isOutput=True)

    # (t, (dd hp), (b n), w); (dd hp) merges to one stride-176 dim, (b n) too
    x_r = x.rearrange("b (n t dd) hp w -> t (dd hp) (b n) w", t=NT, dd=DD)

    with tile.TileContext(nc) as tc, ExitStack() as ctx:
        cpool = ctx.enter_context(tc.tile_pool(name="const", bufs=1))
        xpool = ctx.enter_context(tc.tile_pool(name="xp", bufs=3))
        spool = ctx.enter_context(tc.tile_pool(name="scr", bufs=2))

        meta_sb = cpool.tile([P, 2 * FREE], i8)
        nc.sync.dma_start(meta_sb[:], meta[:])
        dcol_sb = cpool.tile([P, NT], f32)
        nc.sync.dma_start(dcol_sb[:], dcol[:])
        gtc_ap = meta_sb[:, 0:FREE]
        bigw_ap = meta_sb[:, FREE:2 * FREE]

        cols1 = cpool.tile([P, NT], f32)
        cols2 = cpool.tile([P, NT], f32)

        for t in range(reps * NT):
            t = t % NT
            if mask_mode == "dma_accum":
                xm = xpool.tile([P, FREE], f32, tag="xm")
                nc.sync.dma_start(xm[:], x_r[t])
                # xm += bigw via DMA dest-reduce (SBUF->SBUF, no HBM traffic):
                # masks invalid pixels without an extra DVE pass
                nc.gpsimd.dma_start(xm[:], bigw_ap, accum_op=mybir.AluOpType.add)
            else:
                xraw = xpool.tile([P, FREE], f32, tag="xraw")
                nc.sync.dma_start(xraw[:], x_r[t])
                xm = xpool.tile([P, FREE], f32, tag="xm")
                nc.vector.tensor_add(xm[:], xraw[:], bigw_ap)
            sp_scr = spool.tile([P, FREE], f32, tag="sp")
            if softplus_mode == "native":
                nc.scalar.activation(
                    sp_scr[:], xm[:], mybir.ActivationFunctionType.Softplus,
                    accum_out=cols1[:, t:t + 1],
                )
            else:  # ln(1 + exp(x)); exp(-30000) underflows to 0 for masked px
                ex_scr = spool.tile([P, FREE], f32, tag="ex")
                nc.scalar.activation(
                    ex_scr[:], xm[:], mybir.ActivationFunctionType.Exp
                )
                nc.scalar.activation(
                    sp_scr[:], ex_scr[:], mybir.ActivationFunctionType.Ln,
                    bias=1.0, accum_out=cols1[:, t:t + 1],
                )
            st_scr = spool.tile([P, FREE], f32, tag="st")
            nc.vector.scalar_tensor_tensor(
                st_scr[:], gtc_ap, dcol_sb[:, t:t + 1], xm[:],
                op0=mybir.AluOpType.is_equal, op1=mybir.AluOpType.mult,
                accum_out=cols2[:, t:t + 1],
            )

        diff = cpool.tile([P, NT], f32)
        nc.vector.tensor_sub(diff[:], cols1[:], cols2[:])
        red = cpool.tile([P, 1], f32)
        nc.vector.tensor_reduce(
            red[:], diff[:], axis=mybir.AxisListType.X, op=mybir.AluOpType.add
        )
        nc.sync.dma_start(out[:], red[:])

    _split_excess_waits(nc, mybir, limit=1)
    return nc


def _split_excess_waits(nc, mybir, limit=1):
    """walrus core_v2/v3 codegen allows only `limit` fused sem waits per
    instruction; hoist the excess into standalone EventSemaphore waits."""
    fn = nc.m.functions[0]
    for blk in fn.blocks:
        out_instrs = []
        for inst in blk.instructions:
            si = getattr(inst, "sync_info", None)
            waits = list(si.on_wait) if si is not None and si.on_wait else []
            if len(waits) > limit:
                extra, keep = waits[:-limit], waits[-limit:]
                for i in range(0, len(extra), limit):
                    w = mybir.InstEventSemaphore(
                        name=f"{inst.name}_xw{i}", ins=[], outs=[]
                    )
                    w.engine = inst.engine
                    w.sync_info = mybir.SyncInfo(
                        on_wait=extra[i:i + limit], on_update=[]
                    )
                    nc.register_instruction(w)
                    out_instrs.append(w)
                si.on_wait = keep
            out_instrs.append(inst)
        if len(out_instrs) != len(blk.instructions):
            del blk.instructions[:]
            blk.instructions.extend(out_instrs)


def _host_prep(depth_gt, depth):
    """Build the per-core input maps."""
    depth_gt = np.asarray(depth_gt, dtype=np.float32)
    depth = np.asarray(depth, dtype=np.float32)
    assert depth_gt.shape == (B, N, H, W)
    assert depth.shape == (B, N * D, H, W)

    u = (depth_gt - np.float32(2.0)) * np.float32(2.0)  # /0.5 == *2, exact
    idx = np.clip(np.floor(u), 0.0, float(D)).astype(np.float32)
    invalid = depth_gt == 0.0
    gtc = np.where(invalid, np.float32(INVALID_IDX), idx).astype(np.float32)
    bigw = np.where(invalid, np.float32(BIG_NEG), np.float32(0.0)).astype(np.float32)
    gb = np.stack([gtc.reshape(BN, H, W), bigw.reshape(BN, H, W)])  # (2,BN,H,W)

    pvals = np.arange(P) // HSH
    dcol = (np.arange(NT)[None, :] * DD + pvals[:, None]).astype(np.float32)

    in_maps = []
    for c in range(M):
        h0 = c * HSH
        # (P, 2, BN*W): replicate the (hp) block across the 16 dd partitions
        gb_c = gb[:, :, h0:h0 + HSH, :].transpose(2, 0, 1, 3)  # (HSH,2,BN,W)
        gb_c = np.broadcast_to(gb_c[None], (DD, HSH, 2, BN, W))
        in_maps.append({
            "x": np.ascontiguousarray(depth[:, :, h0:h0 + HSH, :]),
            "meta": np.ascontiguousarray(
                gb_c.reshape(P, 2 * FREE).astype(np.int8)
            ),
            "dcol": dcol,
        })
    return in_maps


def kernel(depth_gt, depth):
    from concourse.bass_utils import run_bass_kernel_spmd

    if "nc" not in _CACHE:
        _CACHE["nc"] = _build_bass()
    nc = _CACHE["nc"]

    in_maps = _host_prep(depth_gt, depth)
    res = run_bass_kernel_spmd(nc, in_maps, list(range(M)))
    total = float(np.sum([r["out"].astype(np.float64).sum() for r in res.results]))
    return np.float32(3.0 * total / NUMEL)


# revision 32
# speedup vs baseline: 1.2929x; 1.2929x over previous
"""Trainium2 Bass kernel for DepthLossForImgBEV (weighted one-hot depth BCE).

Math: with x = raw logits (B,N,D,H,W), gt = depth_gt (B,N,H,W):
  bce(x, t) = softplus(x) - t*x          (t = one-hot(idx); the -100 clamp in
                                          the reference never fires for |x|<100)
  loss = 3.0 * sum_{valid px} [ sum_d softplus(x) - x[idx] ] / (B*N*D*H*W)

Device computation per core (shard = 8 of 64 H-rows):
  - layout: partitions = (16 depth-bins x 8 h-rows), free = (12 cameras x 176 w)
  - pass 1 (DVE): xm = x + bigw   (bigw = -80 at invalid pixels, 0 else)
  - pass 2 (ACT): softplus(xm) = ln(1+exp(xm)) with fused accum_out
                  (invalid pixels give softplus(x-80) = 0 exactly in f32)
  - pass 3 (DVE): scalar_tensor_tensor (gtc == d) * xm with fused accum_out
                  (gtc = bucket idx, or 120 at invalid pixels -> no match)
  - epilogue: cols1 - cols2, reduce to (128,1), DMA out.
Host: sum the 8 per-core (128,1) partials, scale by 3/numel.

Measured on trn2 (8 cores, axon): steady-state 19.4 us/pass per core
(7.57 MB logits + 1.08 MB bf16 meta per core ~ HBM streaming roofline);
rel err vs fp32 jax reference: 8.6e-7.

Notes from tuning:
  - walrus core_v2/v3 codegen accepts only ONE fused sem wait per
    instruction on this toolchain -> _split_excess_waits hoists extras
    into standalone EventSemaphore instructions.
  - gpsimd dest-reduce DMA (accum_op=add) mis-executes on HW here
    (+0.9% error; mask_mode="dma_accum" kept for reference, do not use).
  - native Softplus is not in this compiler's act tables; exp+ln live in
    one table (no reload thrash).
  - int8 meta was tried: DVE mixed-dtype reads made it slower than bf16.
"""

import numpy as np

B, N, D, H, W = 2, 6, 112, 64, 176
M = 8  # cores
HSH = H // M  # 8 h-rows per core
DD = 16  # depth bins per partition block
NT = D // DD  # 7 tiles
BN = B * N  # 12
P = 128
FREE = BN * W  # 2112
NUMEL = B * N * D * H * W
INVALID_IDX = 120.0  # any value outside [0,112]
# mask offset: large enough that ln(1+exp(x+BIG_NEG)) == 0 exactly in f32
# (1 + 2e-33 rounds to 1.0), small enough to stay inside the ACT exp LUT's
# valid input range (~[-87, 88]) — a -30000 mask hit LUT wraparound garbage
BIG_NEG = -80.0

_CACHE = {}


def _build_bass(softplus_mode="exp_ln", mask_mode="dve_add", reps=1):
    from contextlib import ExitStack

    import concourse.bass as bass
    import concourse.mybir as mybir
    import concourse.tile as tile

    f32 = mybir.dt.float32
    nc = bass.Bass()

    bf16 = mybir.dt.bfloat16
    x = nc.declare_dram_parameter("x", [B, N * D, HSH, W], f32, isOutput=False)
    # meta[p] = [gtc (FREE) | bigw (FREE)] in bf16 (all values exact),
    # host pre-replicated across the 16 depth-bin partition blocks
    meta = nc.declare_dram_parameter("meta", [P, 2 * FREE], bf16, isOutput=False)
    dcol = nc.declare_dram_parameter("dcol", [P, NT], f32, isOutput=False)
    out = nc.declare_dram_parameter("out", [P, 1], f32, isOutput=True)

    # (t, (dd hp), (b n), w); (dd hp) merges to one stride-176 dim, (b n) too
    x_r = x.rearrange("b (n t dd) hp w -> t (dd hp) (b n) w", t=NT, dd=DD)

    with tile.TileContext(nc) as tc, ExitStack() as ctx:
        cpool = ctx.enter_context(tc.tile_pool(name="const", bufs=1))
        xpool = ctx.enter_context(tc.tile_pool(name="xp", bufs=3))
        spool = ctx.enter_context(tc.tile_pool(name="scr", bufs=2))

        meta_sb = cpool.tile([P, 2 * FREE], bf16)
        nc.sync.dma_start(meta_sb[:], meta[:])
        dcol_sb = cpool.tile([P, NT], f32)
        nc.sync.dma_start(dcol_sb[:], dcol[:])
        gtc_ap = meta_sb[:, 0:FREE]
        bigw_ap = meta_sb[:, FREE:2 * FREE]

        cols1 = cpool.tile([P, NT], f32)
        cols2 = cpool.tile([P, NT], f32)

        for t in range(reps * NT):
            t = t % NT
            if mask_mode == "dma_accum":
                xm = xpool.tile([P, FREE], f32, tag="xm")
                nc.sync.dma_start(xm[:], x_r[t])
                # xm += bigw via DMA dest-reduce (SBUF->SBUF, no HBM traffic):
                # masks invalid pixels without an extra DVE pass
                nc.gpsimd.dma_start(xm[:], bigw_ap, accum_op=mybir.AluOpType.add)
            else:
                xraw = xpool.tile([P, FREE], f32, tag="xraw")
                nc.sync.dma_start(xraw[:], x_r[t])
                xm = xpool.tile([P, FREE], f32, tag="xm")
                nc.vector.tensor_add(xm[:], xraw[:], bigw_ap)
            sp_scr = spool.tile([P, FREE], f32, tag="sp")
            if softplus_mode == "native":
                nc.scalar.activation(
                    sp_scr[:], xm[:], mybir.ActivationFunctionType.Softplus,
                    accum_out=cols1[:, t:t + 1],
                )
            else:  # ln(1 + exp(x)); exp(-30000) underflows to 0 for masked px
                ex_scr = spool.tile([P, FREE], f32, tag="ex")
                nc.scalar.activation(
                    ex_scr[:], xm[:], mybir.ActivationFunctionType.Exp
                )
                nc.scalar.activation(
                    sp_scr[:], ex_scr[:], mybir.ActivationFunctionType.Ln,
                    bias=1.0, accum_out=cols1[:, t:t + 1],
                )
            st_scr = spool.tile([P, FREE], f32, tag="st")
            nc.vector.scalar_tensor_tensor(
                st_scr[:], gtc_ap, dcol_sb[:, t:t + 1], xm[:],
                op0=mybir.AluOpType.is_equal, op1=mybir.AluOpType.mult,
                accum_out=cols2[:, t:t + 1],
            )

        diff = cpool.tile([P, NT], f32)
        nc.vector.tensor_sub(diff[:], cols1[:], cols2[:])
        red = cpool.tile([P, 1], f32)
        nc.vector.tensor_reduce(
            red[:], diff[:], axis=mybir.AxisListType.X, op=mybir.AluOpType.add
        )
        nc.sync.dma_start(out[:], red[:])

    _split_excess_waits(nc, mybir, limit=1)
    return nc


def _split_excess_waits(nc, mybir, limit=1):
    """walrus core_v2/v3 codegen allows only `limit` fused sem waits per
    instruction; hoist the excess into standalone EventSemaphore waits."""
    fn = nc.m.functions[0]
    for blk in fn.blocks:
        out_instrs = []
        for inst in blk.instructions:
            si = getattr(inst, "sync_info", None)
            waits = list(si.on_wait) if si is not None and si.on_wait else []
            if len(waits) > limit:
                extra, keep = waits[:-limit], waits[-limit:]
                for i in range(0, len(extra), limit):
                    w = mybir.InstEventSemaphore(
                        name=f"{inst.name}_xw{i}", ins=[], outs=[]
                    )
                    w.engine = inst.engine
                    w.sync_info = mybir.SyncInfo(
                        on_wait=extra[i:i + limit], on_update=[]
                    )
                    nc.register_instruction(w)
                    out_instrs.append(w)
                si.on_wait = keep
            out_instrs.append(inst)
        if len(out_instrs) != len(blk.instructions):
            del blk.instructions[:]
            blk.instructions.extend(out_instrs)


def _host_prep(depth_gt, depth):
    """Build the per-core input maps."""
    import ml_dtypes
    depth_gt = np.asarray(depth_gt, dtype=np.float32)
    depth = np.asarray(depth, dtype=np.float32)
    assert depth_gt.shape == (B, N, H, W)
    assert depth.shape == (B, N * D, H, W)

    u = (depth_gt - np.float32(2.0)) * np.float32(2.0)  # /0.5 == *2, exact
    idx = np.clip(np.floor(u), 0.0, float(D)).astype(np.float32)
    invalid = depth_gt == 0.0
    gtc = np.where(invalid, np.float32(INVALID_IDX), idx).astype(np.float32)
    bigw = np.where(invalid, np.float32(BIG_NEG), np.float32(0.0)).astype(np.float32)
    gb = np.stack([gtc.reshape(BN, H, W), bigw.reshape(BN, H, W)])  # (2,BN,H,W)

    pvals = np.arange(P) // HSH
    dcol = (np.arange(NT)[None, :] * DD + pvals[:, None]).astype(np.float32)

    in_maps = []
    for c in range(M):
        h0 = c * HSH
        # (P, 2, BN*W): replicate the (hp) block across the 16 dd partitions
        gb_c = gb[:, :, h0:h0 + HSH, :].transpose(2, 0, 1, 3)  # (HSH,2,BN,W)
        gb_c = np.broadcast_to(gb_c[None], (DD, HSH, 2, BN, W))
        in_maps.append({
            "x": np.ascontiguousarray(depth[:, :, h0:h0 + HSH, :]),
            "meta": np.ascontiguousarray(
                gb_c.reshape(P, 2 * FREE).astype(ml_dtypes.bfloat16)
            ),
            "dcol": dcol,
        })
    return in_maps


def kernel(depth_gt, depth):
    from concourse.bass_utils import run_bass_kernel_spmd

    if "nc" not in _CACHE:
        _CACHE["nc"] = _build_bass()
    nc = _CACHE["nc"]

    in_maps = _host_prep(depth_gt, depth)
    res = run_bass_kernel_spmd(nc, in_maps, list(range(M)))
    total = float(np.sum([r["out"].astype(np.float64).sum() for r in res.results]))
    return np.float32(3.0 * total / NUMEL)


# revision 40
# speedup vs baseline: 1.8090x; 1.3991x over previous
"""Trainium2 Bass kernel for DepthLossForImgBEV (weighted one-hot depth BCE).

Math: with x = raw logits (B,N,D,H,W), gt = depth_gt (B,N,H,W):
  bce(x, t) = softplus(x) - t*x          (t = one-hot(idx); the -100 clamp in
                                          the reference never fires for |x|<100)
  loss = 3.0 * sum_{valid px} [ sum_d softplus(x) - x[idx] ] / (B*N*D*H*W)

Device computation per core (shard = 8 of 64 H-rows):
  - layout: partitions = (16 depth-bins x 8 h-rows), free = (12 cameras x 176 w)
  - pass 1 (DVE): xm = x + bigw   (bigw = -80 at invalid pixels, 0 else)
  - pass 2 (ACT): softplus(xm) = ln(1+exp(xm)) with fused accum_out
                  (invalid pixels give softplus(x-80) = 0 exactly in f32)
  - pass 3 (DVE): scalar_tensor_tensor (gtc == d) * xm with fused accum_out
                  (gtc = bucket idx, or 120 at invalid pixels -> no match)
  - epilogue: cols1 - cols2, reduce to (128,1), DMA out.
Host: sum the 8 per-core (128,1) partials, scale by 3/numel.

Measured on trn2 (8 cores, axon): steady-state 17.6 us/pass per core for
7.57 MB of logits (pure-DMA floor measured 14.2 us = 532 GB/s/core);
rel err vs fp32 jax reference: 8.6e-7.

Notes from tuning (each measured via reps-slope on HW):
  - walrus core_v2/v3 codegen accepts only ONE fused sem wait per
    instruction on this toolchain -> _split_excess_waits hoists extras
    into standalone EventSemaphore instructions.
  - gpsimd dest-reduce DMA (accum_op=add) mis-executes on HW here
    (+0.9% error; do not use).
  - native Softplus is not in this compiler's act tables; exp+ln live in
    one table (no reload thrash).
  - pairing depth-bin tiles into (128, 4224) ACT/DVE instructions +
    xpool bufs=3 cut 19.4 -> 17.6 us (per-instr overhead + DMA lookahead).
  - losers: int8 meta (24.5), bf16 xm (18.0), per-tile non-broadcast adds
    (19.6), bufs 4/3 (20.1), dma_accum masking (wrong results).
"""

import numpy as np

B, N, D, H, W = 2, 6, 112, 64, 176
M = 8  # cores
HSH = H // M  # 8 h-rows per core
DD = 16  # depth bins per partition block
NT = D // DD  # 7 tiles
BN = B * N  # 12
P = 128
FREE = BN * W  # 2112
NUMEL = B * N * D * H * W
INVALID_IDX = 120.0  # any value outside [0,112]
# mask offset: large enough that ln(1+exp(x+BIG_NEG)) == 0 exactly in f32
# (1 + 2e-33 rounds to 1.0), small enough to stay inside the ACT exp LUT's
# valid input range (~[-87, 88]) — a -30000 mask hit LUT wraparound garbage
BIG_NEG = -80.0

_CACHE = {}


def _build_bass(softplus_mode="exp_ln", mask_mode="dve_add", reps=1,
                drop_stt=False, drop_ln=False, dma_only=False, xm_bf16=False,
                add_no_bcast=False):
    from contextlib import ExitStack

    import concourse.bass as bass
    import concourse.mybir as mybir
    import concourse.tile as tile

    f32 = mybir.dt.float32
    nc = bass.Bass()

    bf16 = mybir.dt.bfloat16
    x = nc.declare_dram_parameter("x", [B, N * D, HSH, W], f32, isOutput=False)
    # meta[p] = [gtc (FREE) | bigw (FREE)] in bf16 (all values exact),
    # host pre-replicated across the 16 depth-bin partition blocks
    meta = nc.declare_dram_parameter("meta", [P, 2 * FREE], bf16, isOutput=False)
    dcol = nc.declare_dram_parameter("dcol", [P, NT], f32, isOutput=False)
    out = nc.declare_dram_parameter("out", [P, 1], f32, isOutput=True)

    # (t, (dd hp), (b n), w); (dd hp) merges to one stride-176 dim, (b n) too
    x_r = x.rearrange("b (n t dd) hp w -> t (dd hp) (b n) w", t=NT, dd=DD)

    # group the 7 depth-bin tiles into pairs for the elementwise passes —
    # halves ACT/DVE per-instruction overhead; accumulator granularity is
    # irrelevant because every column is summed at the end anyway
    groups = [(0, 1), (2, 3), (4, 5), (6,)]
    NG = len(groups)

    with tile.TileContext(nc) as tc, ExitStack() as ctx:
        cpool = ctx.enter_context(tc.tile_pool(name="const", bufs=1))
        xpool = ctx.enter_context(tc.tile_pool(name="xp", bufs=3))
        spool = ctx.enter_context(tc.tile_pool(name="scr", bufs=2))

        meta_sb = cpool.tile([P, 2 * FREE], bf16)
        nc.sync.dma_start(meta_sb[:], meta[:])
        dcol_sb = cpool.tile([P, NT], f32)
        nc.sync.dma_start(dcol_sb[:], dcol[:])
        gtc_ap = meta_sb[:, 0:FREE]
        bigw_ap = meta_sb[:, FREE:2 * FREE]

        cols1 = cpool.tile([P, reps * NG], f32)
        cols2 = cpool.tile([P, reps * NT], f32)

        for rep in range(reps):
            for gi, g in enumerate(groups):
                L = len(g)
                xraw = xpool.tile([P, 2, FREE], f32, tag="xraw")
                for j, t in enumerate(g):
                    nc.sync.dma_start(xraw[:, j], x_r[t])
                if dma_only:
                    continue
                xm = xpool.tile([P, 2, FREE], bf16 if xm_bf16 else f32,
                                tag="xm")
                if add_no_bcast:
                    for j in range(L):
                        nc.vector.tensor_add(xm[:, j], xraw[:, j], bigw_ap)
                else:
                    bigw_b = bigw_ap.unsqueeze(1).broadcast_to([P, L, FREE])
                    nc.vector.tensor_add(xm[:, :L], xraw[:, :L], bigw_b)
                # gather: on bf16 xm (2x DVE mode) or raw f32 x
                if not drop_stt:
                    for j, t in enumerate(g):
                        gsrc = xm[:, j] if xm_bf16 else xraw[:, j]
                        st_scr = spool.tile([P, FREE],
                                            bf16 if xm_bf16 else f32, tag="st")
                        nc.vector.scalar_tensor_tensor(
                            st_scr[:], gtc_ap, dcol_sb[:, t:t + 1], gsrc,
                            op0=mybir.AluOpType.is_equal,
                            op1=mybir.AluOpType.mult,
                            accum_out=cols2[:, rep * NT + t:rep * NT + t + 1],
                        )
                c1 = cols1[:, rep * NG + gi:rep * NG + gi + 1]
                if drop_ln:  # timing diagnostic only: 1 ACT pass
                    sp_scr = spool.tile([P, 2, FREE], f32, tag="sp")
                    nc.scalar.activation(
                        sp_scr[:, :L], xm[:, :L],
                        mybir.ActivationFunctionType.Exp, accum_out=c1,
                    )
                else:  # softplus = ln(1 + exp(xm)); masked px underflow to 0
                    ex_scr = spool.tile([P, 2, FREE], f32, tag="ex")
                    nc.scalar.activation(
                        ex_scr[:, :L], xm[:, :L],
                        mybir.ActivationFunctionType.Exp,
                    )
                    sp_scr = spool.tile([P, 2, FREE], f32, tag="sp")
                    nc.scalar.activation(
                        sp_scr[:, :L], ex_scr[:, :L],
                        mybir.ActivationFunctionType.Ln, bias=1.0, accum_out=c1,
                    )

        if dma_only:
            zcol = cpool.tile([P, 1], f32)
            nc.vector.memset(zcol[:], 0.0)
            nc.sync.dma_start(out[:], zcol[:])
        else:
            r1 = cpool.tile([P, 1], f32)
            nc.vector.tensor_reduce(
                r1[:], cols1[:], axis=mybir.AxisListType.X,
                op=mybir.AluOpType.add,
            )
            red = cpool.tile([P, 1], f32)
            if drop_stt:
                nc.vector.tensor_copy(red[:], r1[:])
            else:
                r2 = cpool.tile([P, 1], f32)
                nc.vector.tensor_reduce(
                    r2[:], cols2[:], axis=mybir.AxisListType.X,
                    op=mybir.AluOpType.add,
                )
                nc.vector.tensor_sub(red[:], r1[:], r2[:])
            nc.sync.dma_start(out[:], red[:])

    _split_excess_waits(nc, mybir, limit=1)
    return nc


def _split_excess_waits(nc, mybir, limit=1):
    """walrus core_v2/v3 codegen allows only `limit` fused sem waits per
    instruction; hoist the excess into standalone EventSemaphore waits."""
    fn = nc.m.functions[0]
    for blk in fn.blocks:
        out_instrs = []
        for inst in blk.instructions:
            si = getattr(inst, "sync_info", None)
            waits = list(si.on_wait) if si is not None and si.on_wait else []
            if len(waits) > limit:
                extra, keep = waits[:-limit], waits[-limit:]
                for i in range(0, len(extra), limit):
                    w = mybir.InstEventSemaphore(
                        name=f"{inst.name}_xw{i}", ins=[], outs=[]
                    )
                    w.engine = inst.engine
                    w.sync_info = mybir.SyncInfo(
                        on_wait=extra[i:i + limit], on_update=[]
                    )
                    nc.register_instruction(w)
                    out_instrs.append(w)
                si.on_wait = keep
            out_instrs.append(inst)
        if len(out_instrs) != len(blk.instructions):
            del blk.instructions[:]
            blk.instructions.extend(out_instrs)


def _host_prep(depth_gt, depth):
    """Build the per-core input maps."""
    import ml_dtypes
    depth_gt = np.asarray(depth_gt, dtype=np.float32)
    depth = np.asarray(depth, dtype=np.float32)
    assert depth_gt.shape == (B, N, H, W)
    assert depth.shape == (B, N * D, H, W)

    u = (depth_gt - np.float32(2.0)) * np.float32(2.0)  # /0.5 == *2, exact
    idx = np.clip(np.floor(u), 0.0, float(D)).astype(np.float32)
    invalid = depth_gt == 0.0
    gtc = np.where(invalid, np.float32(INVALID_IDX), idx).astype(np.float32)
    bigw = np.where(invalid, np.float32(BIG_NEG), np.float32(0.0)).astype(np.float32)
    gb = np.stack([gtc.reshape(BN, H, W), bigw.reshape(BN, H, W)])  # (2,BN,H,W)

    pvals = np.arange(P) // HSH
    dcol = (np.arange(NT)[None, :] * DD + pvals[:, None]).astype(np.float32)

    in_maps = []
    for c in range(M):
        h0 = c * HSH
        # (P, 2, BN*W): replicate the (hp) block across the 16 dd partitions
        gb_c = gb[:, :, h0:h0 + HSH, :].transpose(2, 0, 1, 3)  # (HSH,2,BN,W)
        gb_c = np.broadcast_to(gb_c[None], (DD, HSH, 2, BN, W))
        in_maps.append({
            "x": np.ascontiguousarray(depth[:, :, h0:h0 + HSH, :]),
            "meta": np.ascontiguousarray(
                gb_c.reshape(P, 2 * FREE).astype(ml_dtypes.bfloat16)
            ),
            "dcol": dcol,
        })
    return in_maps


def kernel(depth_gt, depth):
    from concourse.bass_utils import run_bass_kernel_spmd

    if "nc" not in _CACHE:
        _CACHE["nc"] = _build_bass()
    nc = _CACHE["nc"]

    in_maps = _host_prep(depth_gt, depth)
    res = run_bass_kernel_spmd(nc, in_maps, list(range(M)))
    total = float(np.sum([r["out"].astype(np.float64).sum() for r in res.results]))
    return np.float32(3.0 * total / NUMEL)
